# revision 49
# baseline (speedup 1.0000x reference)
"""Trainium2 Bass kernel for fused LN -> QKV -> (K^T V softmax) linear-attention -> out-proj + residual.

Algebraic restructure vs the direct formulation (kt_v is per-head 64x64 over
the whole sequence, so the K/V path funnels through small weight-side
products):

    xn   = (x - mu) / sigma                    (ln_g folded into the weights)
    G    = xn^T xn                             (1024x1024 Gram, contract tokens)
    ktv  = Wk^T G Wv        per head h: ktv_h = Wk_h^T G Wv_h    (linear in G)
    s    = softmax(ktv, axis=e)
    W3   = Wq @ diag(1/sums) @ blockdiag(exp) @ Wout
    out  = xn @ W3 + (xn*sigma + mu)           (residual rebuilt, not re-read)

K and V activations are never materialized: G costs half the K,V projection
and ktv is tiny.

Schedule (built from trace evidence; LNC1 pairs share an HBM port, so the
per-core stream is ~179GB/s and phases are laid out around DMA feasibility):
  - Phase A streams fp16 x only (LN + Gram passes); the fp16 wk/wv stream is
    gated into the Gram-pass-1/A3 window, which has no HBM traffic.
  - A3 computes A = G@Wv in four 2-column-block groups, ct-sequential so the
    wv tiles are consumed in DMA arrival order; the previous group's 16
    short ktv matmuls hide inside each group's 32 long ones.
  - The pairwise ktv exchange is an AllGather (4-9us; AllReduce measures
    14-27us here) + a local one-op vector add.
  - The xnT transposes (LDWEIGHTS-bound, 2-byte via fp16) and N_DUMMY dead
    warm-keeper matmuls cover the exchange; transpose-mode does not count as
    PE-busy for the HAM clock gate, so the dummies keep the 2.4GHz clock for
    the bf16 tail (W2 -> W3 -> out), which runs at the 216ns/512-col roofline.

Precision: fp16 x/xn/G/A/Wk/Wv (the softmax logits have std ~64 and are
hypersensitive, but fp16 keeps the absolute logit noise ~0.1); softmax 1/sum
is folded into the W2 PSUM drain as a per-partition scale; the post-softmax
chain and output are bf16. Measured rel_l2 ~5e-3 (tolerance 2e-2).

Sharding: data-parallel, 8 shards of 2048 tokens. Cores 2b, 2b+1 hold batch
element b; ktv partials (linear in G) are exchanged pairwise, everything
else is local.
"""

import numpy as np

# Problem shapes (hardcoded per harness contract).
B, L, D = 4, 4096, 1024
H, HD = 16, 64
NCORES = 8
TOK = B * L // NCORES  # 2048 tokens per core
P = 128
NT = TOK // P  # 16 token tiles per core
NC_ = D // P  # 8 channel tiles
EPS = 1e-5
# Warm-keeper matmuls between the xnT transposes and the collective-gated
# tail; sized to bridge the small gap until the exchange result lands.
N_DUMMY = 45


def _build(tc, nc, mybir, x_ap, wk_ap, wv_ap, wqT_ap, wout_ap, out_ap,
           use_collective=True):
    from concourse.masks import make_identity

    f32 = mybir.dt.float32
    f32r = mybir.dt.float32r
    bf16 = mybir.dt.bfloat16
    f16 = mybir.dt.float16
    AF = mybir.ActivationFunctionType
    OP = mybir.AluOpType

    def popen(name, bufs, space="SBUF"):
        cm = tc.tile_pool(name=name, bufs=bufs, space=space)
        return cm, cm.__enter__()

    def pclose(cm):
        cm.__exit__(None, None, None)

    consts_cm, consts = popen("consts", 1)
    smpool_cm, smpool = popen("smpool", 1)
    dram_cm, dram = popen("dram", 1, space="DRAM")
    xn_cm, xn_pool = popen("xn_pool", 1)

    identf16 = consts.tile([P, P], f16)
    make_identity(nc, identf16)
    ident16 = consts.tile([P, P], bf16)
    make_identity(nc, ident16)
    eps_t = consts.tile([P, 1], f32)
    nc.vector.memset(eps_t, EPS)

    # Pre-zeroed block-diag staging tiles for the softmax transposes (filled
    # with the two 64x64 diagonal blocks per head pair after the collective).
    sblk = [smpool.tile([P, P], bf16, name=f"sblk{p}") for p in range(NC_)]
    for p in range(NC_):
        nc.vector.memset(sblk[p], 0.0)

    # Warm-up collective: absorbs the mesh/staging setup cost so the real
    # ktv AllGather later starts hot.
    warm_sb = consts.tile([P, 512], f32)
    nc.vector.memset(warm_sb, 0.0)
    warm_in16 = dram.tile([P, 512], f16, name="warm_in16")
    warm_ag_out = dram.tile([P, 1024], f16, name="warm_ag_out")
    nc.gpsimd.dma_start(out=warm_in16, in_=warm_sb.bitcast(f16)[:, 0:512])
    if use_collective:
        nc.gpsimd.collective_compute(
            "AllGather",
            mybir.AluOpType.bypass,
            ins=[warm_in16.opt()],
            outs=[warm_ag_out.opt()],
            replica_groups=[[0, 1], [2, 3], [4, 5], [6, 7]],
        )

    # Persistent f16 xn (token-major) for the Gram path. fp16 keeps the
    # hypersensitive ktv-logit path within ~0.1 absolute noise (vs bf16's
    # ~0.3) while halving SBUF/DMA traffic and making the xnT transposes
    # 2-byte.
    xn = [xn_pool.tile([P, D], f16, tag=f"xn{i}", name=f"xn{i}")
          for i in range(NT)]

    # Persistent per-tile (mu, sigma): phase D rebuilds the residual as
    # x = xn*sigma + mu instead of re-reading 8MB of x from HBM. Opened
    # here so it outlives the mid-kernel pool closes (frees are stack-LIFO).
    msd_cm, msd_pool = popen("msd_pool", 1)
    msd = [msd_pool.tile([P, 2], f32, tag=f"msd{i}", name=f"msd{i}")
           for i in range(NT)]

    # wqT/wout live for the whole kernel; their DMAs are chained behind the
    # wk stream (see weight_dma) so all weight traffic lands in the A3
    # window and the HBM port + SDMA engines are QUIET when the ktv
    # AllGather fires (concurrent wqT/wout DMAs measured as the real
    # exchange running 2x slower than the warmup one).
    wqT_cm, wqT_pool = popen("wqT_pool", 1)
    wout_cm, wout_pool = popen("wout_pool", 1)
    wqT = [wqT_pool.tile([P, D], bf16, tag=f"wqT{i}", name=f"wqT{i}")
           for i in range(NC_)]
    wout = [wout_pool.tile([P, D], bf16, tag=f"wo{i}", name=f"wo{i}")
            for i in range(NC_)]

    # fp16 K/V projection weights and the Gram matrix (freed at the exchange
    # kick, so the bf16 tail reuses their SBUF space). Their DMAs are gated
    # into mid-pass-0 (see weight_dma) so they don't starve the x stream.
    wkv_cm, wkv_pool = popen("wkv_pool", 1)
    gsb_cm, gsb_pool = popen("gsb_pool", 1)
    g_sb = [gsb_pool.tile([P, D], f16, tag=f"g{i}", name=f"gsb{i}")
            for i in range(NC_)]
    wk = [wkv_pool.tile([P, D], f16, tag=f"wk{i}", name=f"wk{i}")
          for i in range(NC_)]
    wv = [wkv_pool.tile([P, D], f16, tag=f"wv{i}", name=f"wv{i}")
          for i in range(NC_)]

    # Scratch used as a scheduling gate for the weight DMAs (see below).
    wgate = consts.tile([P, 1], f32)

    def weight_dma(tt):
        # Under LNC1 the two cores of a pair share one HBM port, so weights
        # at t=0 would starve the x stream feeding LN. Gated behind tile 6's
        # LN output (a gpsimd copy dep), the 4MB of fp16 weights fill the
        # x-stream's leftover bandwidth plus the Gram-pass-1 window and land
        # just before A3 consumes them (all wv first, then wk by cb).
        if tt == 6:
            nc.gpsimd.tensor_copy(out=wgate, in_=xn[6][:, 0:1])
            for i in range(NC_):
                nc.gpsimd.dma_start(out=wv[i], in_=wv_ap[i * P:(i + 1) * P, :])
            for i in range(NC_):
                nc.gpsimd.dma_start(out=wk[i], in_=wk_ap[i * P:(i + 1) * P, :])
            for i in range(NC_):
                nc.gpsimd.dma_start(out=wqT[i], in_=wqT_ap[i * P:(i + 1) * P, :])
            for i in range(NC_):
                nc.gpsimd.dma_start(out=wout[i], in_=wout_ap[i * P:(i + 1) * P, :])

    # ---- Phase A: LN -> xn (f32) + Gram accumulation ----
    # G needs 16 PSUM banks; ping-pong two 3-bank tag sets (6 banks + mm's 2)
    # across 6 passes over the resident xn tiles so pass N+1 never waits on
    # pass N's drains.
    a_cm, a_pools = zip(*[popen("xpool", 5), popen("stpool", 8)])
    xpool, stpool = a_pools
    g_ps_cm, g_psum = popen("g_psum", 1, space="PSUM")

    def ln_tile(tt):
        x_t = xpool.tile([P, D], f16, tag="x", name="x_t")
        eng = nc.scalar if tt in (1, 3) else nc.sync
        eng.dma_start(out=x_t, in_=x_ap[tt * P:(tt + 1) * P, :])
        stats = stpool.tile([P, 2, 6], f32, tag="stats", name="stats")
        nc.vector.bn_stats(out=stats[:, 0, :], in_=x_t[:, 0:512])
        nc.vector.bn_stats(out=stats[:, 1, :], in_=x_t[:, 512:1024])
        mv = stpool.tile([P, 2], f32, tag="mv", name="mv")
        nc.vector.bn_aggr(out=mv, in_=stats)
        sd = msd[tt][:, 1:2]
        nc.scalar.activation(out=sd, in_=mv[:, 1:2], func=AF.Sqrt, bias=eps_t)
        nc.vector.tensor_copy(out=msd[tt][:, 0:1], in_=mv[:, 0:1])
        rstd = stpool.tile([P, 1], f32, tag="rstd", name="rstd")
        nc.vector.reciprocal(out=rstd, in_=sd)
        nmr = stpool.tile([P, 1], f32, tag="nmr", name="nmr")
        nc.vector.tensor_scalar(out=nmr, in0=mv[:, 0:1], scalar1=rstd,
                                scalar2=-1.0, op0=OP.mult, op1=OP.mult)
        for hh in range(2):
            nc.scalar.activation(out=xn[tt][:, hh * 512:(hh + 1) * 512],
                                 in_=x_t[:, hh * 512:(hh + 1) * 512],
                                 func=AF.Identity, scale=rstd, bias=nmr)

    # G is symmetric: compute the top 4 block-rows in full (8 half-blocks,
    # all 8 PSUM banks, overlapped with LN) plus the diagonal quarter
    # (cb 4-7, right half), then mirror the lower-left quarter by
    # PE-transposing G[0:4, 512:1024].
    GROUPS = [[(cb, h) for h in range(2) for cb in range(4)],
              [(cb, 1) for cb in range(4, 8)]]
    for pi, group in enumerate(GROUPS):
        ps = {}
        for si, (cb, h) in enumerate(group):
            slot = si + 4 if pi == 1 else si
            ps[(cb, h)] = g_psum.tile([P, 512], f32, tag=f"gp{slot}",
                                      name=f"gps{cb}_{h}")
        for tt in range(NT):
            if pi == 0:
                if tt == 0:
                    # Front-load tile 0's whole LN chain in the static
                    # schedule: otherwise tiles 1-3's stats interleave into
                    # its dependency chain and delay the first Gram matmul
                    # by ~4us.
                    with tc.high_priority():
                        ln_tile(tt)
                else:
                    ln_tile(tt)
                weight_dma(tt)
            for (cb, h) in group:
                nc.tensor.matmul(ps[(cb, h)], xn[tt][:, cb * P:(cb + 1) * P],
                                 xn[tt][:, h * 512:(h + 1) * 512],
                                 start=(tt == 0), stop=(tt == NT - 1))
        keys = list(ps)
        if pi == 0:
            keys = keys[4:] + keys[:4]  # drain pass-2's reuse slots first
        for i, (cb, h) in enumerate(keys):
            if i % 2 == 0:
                nc.vector.tensor_copy(out=g_sb[cb][:, h * 512:(h + 1) * 512],
                                      in_=ps[(cb, h)])
            else:
                nc.scalar.copy(out=g_sb[cb][:, h * 512:(h + 1) * 512],
                               in_=ps[(cb, h)])

    pclose(g_ps_cm)
    for cm in reversed(a_cm):
        pclose(cm)

    # Mirror: g_sb[4+i][:, 0:512] (4 blocks each) = G[0:4, 512:1024]^T.
    trm_cm, trm_psum = popen("trm_psum", 2, space="PSUM")
    for i in range(4):
        trm = trm_psum.tile([P, 512], f16, tag="trm", name="trm")
        for j in range(4):
            nc.tensor.transpose(
                trm[:, j * P:(j + 1) * P],
                g_sb[j][:, 512 + i * P:512 + (i + 1) * P], identf16)
        if i % 2 == 0:
            nc.vector.tensor_copy(out=g_sb[4 + i][:, 0:512], in_=trm)
        else:
            nc.scalar.copy(out=g_sb[4 + i][:, 0:512], in_=trm)
    pclose(trm_cm)

    # ---- Phase A3 (fused): A = G @ Wv in four 2-column-block groups with
    # ct-sequential accumulation, so wv[ct] tiles are consumed in DMA-arrival
    # order (the weights stream in DURING A3 -- see weight_dma). Each group's
    # 32 long matmuls also hide the previous group's 16 short ktv matmuls
    # (kt chains fold A[cb] over cb; adds commute so group order is free).
    asb_cm, asb_pool = popen("asb_pool", 3)
    ktv_ps_cm, ktv_psum = popen("ktv_psum", 1, space="PSUM")
    a3m_cm, a3m_psum = popen("a3m_psum", 1, space="PSUM")
    a3x_cm, a3x_psum = popen("a3x_psum", 1, space="PSUM")

    # Two head pairs share one [P, 512] PSUM bank; only the very first matmul
    # into a bank sets start=True (marks the whole bank pending-zero, so the
    # second pair's first write is zero-initialized by the hardware).
    kt = [ktv_psum.tile([P, 512], f32, tag=f"kt{q}", name=f"kt{q}")
          for q in range(4)]

    def ktv_mm(cb, i):
        # i-th of the 8 ktv matmuls folding a_t[cb] into the pair chains.
        q, pr = i // 2, i % 2
        p = 2 * q + pr
        nc.tensor.matmul(kt[q][:, pr * 256:(pr + 1) * 256],
                         wk[cb][:, p * P:(p + 1) * P],
                         a_sb[cb][:, (p // 2) * 256:(p // 2 + 1) * 256],
                         start=(cb == 0 and pr == 0),
                         stop=(cb == NC_ - 1 and pr == 1))

    a_sb = {}
    for grp in range(5):
        if grp < 4:
            pair = (2 * grp, 2 * grp + 1)
            ps = {cb: (a3m_psum.tile([P, 512], f32, tag=f"mma{cb % 2}", name="mp0"),
                       a3x_psum.tile([P, 512], f32, tag=f"mmx{cb % 2}", name="mp1"))
                  for cb in pair}
            for ct in range(NC_):
                for cb in pair:
                    nc.tensor.matmul(ps[cb][0], g_sb[ct][:, cb * P:(cb + 1) * P],
                                     wv[ct][:, 0:512],
                                     start=(ct == 0), stop=(ct == NC_ - 1))
                if grp >= 1:
                    ktv_mm(2 * grp - 2, ct)
                for cb in pair:
                    nc.tensor.matmul(ps[cb][1], g_sb[ct][:, cb * P:(cb + 1) * P],
                                     wv[ct][:, 512:1024],
                                     start=(ct == 0), stop=(ct == NC_ - 1))
                if grp >= 1:
                    ktv_mm(2 * grp - 1, ct)
            for cb in pair:
                a_t = asb_pool.tile([P, D], f16, tag=f"a{cb % 2}", name="a_t")
                nc.vector.tensor_copy(out=a_t[:, 0:512], in_=ps[cb][0])
                nc.scalar.copy(out=a_t[:, 512:1024], in_=ps[cb][1])
                a_sb[cb] = a_t
        else:
            for ct in range(NC_):
                ktv_mm(NC_ - 2, ct)
                ktv_mm(NC_ - 1, ct)

    # Stage ktv diag blocks: partition (h%2)*64+d, free (g=h//2, e).
    # Pair p sits in bank q=p//2 at column base (p%2)*256; within its
    # 256-wide quad slice head 2p is at offset (p%2)*128.
    stage = smpool.tile([P, 8, 64], f16, tag="sm864", name="stage")
    for p in range(NC_):
        q, pr = p // 2, p % 2
        off = pr * 256 + pr * 128
        nc.vector.tensor_copy(out=stage[0:64, p, :],
                              in_=kt[q][0:64, off:off + 64])
        nc.scalar.copy(out=stage[64:128, p, :],
                       in_=kt[q][64:128, off + 64:off + 128])

    # Preload the Exp activation table while the collective is in flight --
    # the table swap costs 1.3us and would otherwise land on the post-
    # collective critical path.
    exp_warm = smpool.tile([P, 1], f32, name="exp_warm")
    nc.scalar.activation(out=exp_warm, in_=eps_t, func=AF.Exp)

    # ---- Phase B: exchange ktv partials across the batch pair ----
    # AllGather + local add instead of AllReduce: the pairwise 128KB
    # AllGather measures 4-7us on this fabric vs 14-27us for AllReduce (no
    # CCE reduce phase); the halves are summed locally in one vector op.
    bounce_in = dram.tile([P, 512], f16, name="bounce_in")
    gather_out = dram.tile([2, P, 512], f16, name="gather_out")
    nc.gpsimd.dma_start(out=bounce_in, in_=stage.rearrange("p g e -> p (g e)"))
    if use_collective:
        nc.gpsimd.collective_compute(
            "AllGather",
            mybir.AluOpType.bypass,
            ins=[bounce_in.opt()],
            outs=[gather_out.opt()],
            replica_groups=[[0, 1], [2, 3], [4, 5], [6, 7]],
        )
    else:
        nc.gpsimd.dma_start(out=gather_out[0], in_=bounce_in)
        nc.gpsimd.dma_start(out=gather_out[1], in_=bounce_in)
    kv_both = smpool.tile([P, 2, 512], f16, name="kv_both")
    nc.gpsimd.dma_start(out=kv_both, in_=gather_out.rearrange("s p e -> p s e"))
    kv_red = smpool.tile([P, 8, 64], f32, name="kv_red")
    nc.vector.tensor_add(out=kv_red.rearrange("p g e -> p (g e)"),
                         in0=kv_both[:, 0, :], in1=kv_both[:, 1, :])

    pclose(a3x_cm)
    pclose(a3m_cm)
    pclose(ktv_ps_cm)
    pclose(asb_cm)
    pclose(gsb_cm)
    pclose(wkv_cm)

    # ---- Transposes xn -> xnT (hide the collective) ----
    xnT_cm, xnT_pool = popen("xnT_pool", 1)
    tr_ps_cm, tr_psum = popen("tr_psum", 3, space="PSUM")

    xnT = [xnT_pool.tile([P, TOK], bf16, tag=f"xnT{i}", name=f"xnT{i}")
           for i in range(NC_)]

    # xn is already fp16, so the transposes take it directly (2-byte
    # stationary; no cast pass needed). Drains cast psum f16 -> bf16 xnT.
    for tg in range(NT // 4):
        for ct in range(NC_):
            trt = tr_psum.tile([P, 512], f16, tag="tr", name="trt")
            for i in range(4):
                tt = tg * 4 + i
                nc.tensor.transpose(trt[:, i * P:(i + 1) * P],
                                    xn[tt][:, ct * P:(ct + 1) * P], identf16)
            if ct % 2 == 0:
                nc.vector.tensor_copy(out=xnT[ct][:, tg * 512:(tg + 1) * 512],
                                      in_=trt)
            else:
                nc.scalar.copy(out=xnT[ct][:, tg * 512:(tg + 1) * 512], in_=trt)

    # Warm-keeper matmuls: transpose-mode PE work does not register as
    # "busy" for the HAM clock monitor, so after the transposes the PE clock
    # drops to 1.2GHz and the post-collective tail pays ~5us re-warming.
    # Burn real (dead) N=512 accumulating matmuls sized to bridge the small
    # gap until the AllGather result lands; they run inside the exchange
    # wait so they cost no wall-clock, and the tail then starts at 2.4GHz.
    if N_DUMMY:
        dummy_ps_cm, dummy_psum = popen("dummy_psum", 1, space="PSUM")
        dummy_ps = dummy_psum.tile([P, 512], f32, name="dummy_ps")
        for i in range(N_DUMMY):
            nc.tensor.matmul(dummy_ps, xn[0][:, 0:P], xn[0][:, 0:512],
                             start=(i == 0), stop=(i == N_DUMMY - 1))
        pclose(dummy_ps_cm)

    # ---- Phase C: softmax -> s^T (block-diag pairs) -> W2 -> W3 ----
    # Critical-path-lean layout: exps carry the max-subtract as bias but NOT
    # the accumulator readout (sums come from one grouped vector reduce off
    # the critical path); 1/sum is folded into the W2 PSUM drains (W2 =
    # diag(rinv) @ s~ @ Wout), so the s^T transposes consume the raw exps
    # via pre-zeroed block-diag tiles.
    negmax = smpool.tile([P, 8], f32, name="negmax")
    nc.vector.reduce_max(out=negmax, in_=kv_red, axis=mybir.AxisListType.X,
                         negate=True)
    s_t = smpool.tile([P, 8, 64], bf16, tag="sm864b", name="s_t")
    for g in range(8):
        nc.scalar.activation(out=s_t[:, g, :], in_=kv_red[:, g, :],
                             func=AF.Exp, bias=negmax[:, g:g + 1])
    sums = smpool.tile([P, 8], f32, name="sums")
    nc.vector.reduce_sum(out=sums, in_=s_t, axis=mybir.AxisListType.X)
    rinv = smpool.tile([P, 8], f32, name="rinv")
    nc.vector.reciprocal(out=rinv, in_=sums)

    # Both diag-block fills go on vector: the scalar engine is running the
    # 8 exps at this point and sharing would serialize the critical path.
    sblkT = smpool.tile([P, NC_ * P], bf16, tag="sbT", name="sblkT")
    for sg in range(2):
        trs = tr_psum.tile([P, 512], bf16, tag="trs", name="trs")
        for i in range(4):
            p = sg * 4 + i
            nc.vector.tensor_copy(out=sblk[p][0:64, 0:64], in_=s_t[0:64, p, :])
            nc.vector.tensor_copy(out=sblk[p][64:128, 64:128],
                                  in_=s_t[64:128, p, :])
            nc.tensor.transpose(trs[:, i * P:(i + 1) * P], sblk[p], ident16)
        nc.scalar.copy(out=sblkT[:, sg * 512:(sg + 1) * 512], in_=trs)

    pclose(tr_ps_cm)

    tail_cm, tail_pools = zip(*[
        popen("w2sb_pool", 1), popen("w3sb_pool", 1),
        popen("finm_psum", 8, "PSUM"), popen("xhpool", 3), popen("outpool", 3),
    ])
    w2sb_pool, w3sb_pool, finm_psum, xhpool, outpool = tail_pools

    # W2 = diag(rinv) @ s~ @ Wout; pair p's rows live in wout tile p. The
    # softmax 1/sum lands here as a per-partition scale on the PSUM drain.
    w2_sb = [w2sb_pool.tile([P, D], bf16, tag=f"w2_{i}", name=f"w2_{i}")
             for i in range(NC_)]
    for p in range(NC_):
        mp0 = finm_psum.tile([P, 512], f32, tag="fm", name="mp0")
        mp1 = finm_psum.tile([P, 512], f32, tag="fm", name="mp1")
        sl = sblkT[:, p * P:(p + 1) * P]
        nc.tensor.matmul(mp0, sl, wout[p][:, 0:512], start=True, stop=True)
        nc.tensor.matmul(mp1, sl, wout[p][:, 512:1024], start=True, stop=True)
        nc.vector.tensor_scalar_mul(w2_sb[p][:, 0:512], mp0, rinv[:, p:p + 1])
        nc.scalar.activation(out=w2_sb[p][:, 512:1024], in_=mp1,
                             func=AF.Identity, scale=rinv[:, p:p + 1])

    # W3 = Wq @ W2  (wqT holds Wq^T so hd is the contraction/partition dim).
    w3_sb = [w3sb_pool.tile([P, D], bf16, tag=f"w3_{i}", name=f"w3_{i}")
             for i in range(NC_)]
    for cb in range(NC_):
        mp0 = finm_psum.tile([P, 512], f32, tag="fm", name="mp0")
        mp1 = finm_psum.tile([P, 512], f32, tag="fm", name="mp1")
        for pt in range(NC_):
            lhs = wqT[pt][:, cb * P:(cb + 1) * P]
            nc.tensor.matmul(mp0, lhs, w2_sb[pt][:, 0:512],
                             start=(pt == 0), stop=(pt == NC_ - 1))
            nc.tensor.matmul(mp1, lhs, w2_sb[pt][:, 512:1024],
                             start=(pt == 0), stop=(pt == NC_ - 1))
        nc.vector.tensor_copy(out=w3_sb[cb][:, 0:512], in_=mp0)
        nc.scalar.copy(out=w3_sb[cb][:, 512:1024], in_=mp1)

    # ---- Phase D: out = xn @ W3 + x ----
    # The residual is rebuilt on the scalar engine as x = xn*sigma + mu
    # (saves re-reading 8MB of x from HBM mid-tail); the output is written
    # bf16 (harness tolerance 2e-2 >> bf16 rounding), halving the writeback.
    for tt in range(NT):
        tsl = slice(tt * P, (tt + 1) * P)
        xh = xhpool.tile([P, D], f32, tag="xh", name="xh")
        nc.scalar.activation(out=xh, in_=xn[tt], func=AF.Identity,
                             scale=msd[tt][:, 1:2], bias=msd[tt][:, 0:1])
        out_t = outpool.tile([P, D], bf16, tag="out", name="out_t")
        mp0 = finm_psum.tile([P, 512], f32, tag="fm", name="mp0")
        mp1 = finm_psum.tile([P, 512], f32, tag="fm", name="mp1")
        for ct in range(NC_):
            lhs = xnT[ct][:, tsl]
            nc.tensor.matmul(mp0, lhs, w3_sb[ct][:, 0:512],
                             start=(ct == 0), stop=(ct == NC_ - 1))
            nc.tensor.matmul(mp1, lhs, w3_sb[ct][:, 512:1024],
                             start=(ct == 0), stop=(ct == NC_ - 1))
        nc.vector.tensor_add(out=out_t[:, 0:512], in0=mp0, in1=xh[:, 0:512])
        nc.vector.tensor_add(out=out_t[:, 512:1024], in0=mp1, in1=xh[:, 512:1024])
        nc.sync.dma_start(out=out_ap[tsl, :], in_=out_t)

    for cm in reversed(tail_cm):
        pclose(cm)
    for cm in (xnT_cm, wout_cm, wqT_cm, msd_cm, xn_cm, dram_cm, smpool_cm,
               consts_cm):
        pclose(cm)


_P2P_SEMS = []


def _make_program():
    """Build and compile the SPMD Bass program once."""
    import concourse.bass as bass  # noqa: F401
    import concourse.tile as tile
    from concourse import bacc, bass_interp, mybir
    from contextlib import contextmanager

    @contextmanager
    def _seed_p2p_sems_in_sim():
        # See _build: satisfies the remote-dma handshake sems in the (single
        # core) scheduling simulator only. Hardware semantics are untouched.
        orig = bass_interp.CoreSim.simulate

        def patched(self, *a, **k):
            for num, name in _P2P_SEMS:
                self.update_semaphore(mybir.SyncUpdate(
                    sync_type="semaphore", id=num, ant_name=name,
                    update_mode="sem-add-imm", update_value=16))
            return orig(self, *a, **k)

        bass_interp.CoreSim.simulate = patched
        try:
            yield
        finally:
            bass_interp.CoreSim.simulate = orig

    nc = bacc.Bacc("TRN2", target_bir_lowering=False, debug=False,
                   num_devices=NCORES)
    f16 = mybir.dt.float16
    bf16 = mybir.dt.bfloat16
    x_d = nc.dram_tensor("x_shard", [TOK, D], f16, kind="ExternalInput").ap()
    wk_d = nc.dram_tensor("w_k", [D, D], f16, kind="ExternalInput").ap()
    wv_d = nc.dram_tensor("w_v", [D, D], f16, kind="ExternalInput").ap()
    wqT_d = nc.dram_tensor("w_qT", [D, D], bf16, kind="ExternalInput").ap()
    wout_d = nc.dram_tensor("w_out", [D, D], bf16, kind="ExternalInput").ap()
    out_d = nc.dram_tensor("out_shard", [TOK, D], bf16, kind="ExternalOutput").ap()

    with _seed_p2p_sems_in_sim():
        with tile.TileContext(nc) as tc:
            _build(tc, nc, mybir, x_d, wk_d, wv_d, wqT_d, wout_d, out_d)
    nc.compile()
    return nc


_CACHED_NC = None


def _prepare_inputs(x, w_qkv, b_qkv, w_out, b_out, ln_g, ln_b):
    import ml_dtypes

    bf16 = ml_dtypes.bfloat16
    x = np.ascontiguousarray(np.asarray(x, dtype=np.float32))
    w_qkv = np.asarray(w_qkv, dtype=np.float32)
    b_qkv = np.asarray(b_qkv, dtype=np.float32)
    w_out = np.asarray(w_out, dtype=np.float32)
    b_out = np.asarray(b_out, dtype=np.float32)
    ln_g = np.asarray(ln_g, dtype=np.float32)
    ln_b = np.asarray(ln_b, dtype=np.float32)

    # Fold the LN affine into the QKV projection: xn@W + b with xn = z*g + lb
    # becomes z@(g[:,None]*W) + (b + lb@W).
    w_f = ln_g[:, None] * w_qkv
    b_eff = b_qkv + ln_b @ w_qkv
    if np.abs(b_eff).max() > 0 or np.abs(b_out).max() > 0:
        raise NotImplementedError("nonzero effective biases not supported")

    wqT = np.ascontiguousarray(w_f[:, 0:D].T).astype(bf16)
    wk = np.ascontiguousarray(w_f[:, D:2 * D]).astype(np.float16)
    wv = np.ascontiguousarray(w_f[:, 2 * D:3 * D]).astype(np.float16)
    wout = np.ascontiguousarray(w_out).astype(bf16)

    shards = x.astype(np.float16).reshape(NCORES, TOK, D)
    in_maps = [
        {"x_shard": np.ascontiguousarray(shards[c]), "w_k": wk,
         "w_v": wv, "w_qT": wqT, "w_out": wout}
        for c in range(NCORES)
    ]
    return in_maps


def _run(inputs, trace=False):
    global _CACHED_NC
    from concourse.bass_utils import run_bass_kernel_spmd

    in_maps = _prepare_inputs(**inputs)
    if _CACHED_NC is None:
        _CACHED_NC = _make_program()
    res = run_bass_kernel_spmd(
        _CACHED_NC, in_maps, core_ids=list(range(NCORES)), trace=trace,
    )
    out = np.empty((B, L, D), dtype=np.float32)
    flat = out.reshape(NCORES, TOK, D)
    for c in range(NCORES):
        flat[c] = res.results[c]["out_shard"]
    return out, res


def kernel(**inputs):
    out, _ = _run(inputs, trace=False)
    return out

